# revision 16
# baseline (speedup 1.0000x reference)
"""Multi-head attention Bass kernel for Trainium2 (8 NeuronCores).

Problem: B=2, N=4096, E=768, H=12 heads of dim 64 (nn_MultiHeadAttention).
Sharding: 2 batches x 4 head-groups (3 heads each) = 8 cores.

v2 design (mixed precision, three-engine softmax):
  - QKV projection in bf16 (x, w_qkv bf16; fp32 PSUM accumulation).
  - Q/K/V stored fp8e4m3 in SBUF; scores and P@V matmuls run in fp8
    DoubleRow perf mode (0.5 cycles/row): scores use a zeroed second
    K-slot pair, P@V pairs two kv-blocks per instruction.
  - exp(s/8) is computed by THREE engines in parallel: ACT does exact
    Exp -> fp8 writes; DVE and Pool (gpsimd) use a one-pass Schraudolph:
    int8 bits = trunc(s*log2(e) + 56.15) reinterpreted as fp8e4m3.
  - softmax denominators via a ones-column appended to V (65th row of
    the P@V output); normalization = partition_broadcast + divide.
  - output projection in bf16 against w_proj rows; partial [N, E]
    written as bf16; host sums the 4 partials and adds bias.

Bias handling (exact algebra): K bias drops out of softmax; V bias is
folded into b_proj on the host; Q bias added on-device (fp32) before
the fp8 quantization of Q.
"""

import sys

sys.path.insert(0, "/opt/trn_rl_repo")

import numpy as np
import ml_dtypes

import concourse.bass as bass  # noqa: E402
import concourse.mybir as mybir  # noqa: E402
import concourse.tile as tile  # noqa: E402
from concourse import bacc  # noqa: E402
from concourse.bass_utils import run_bass_kernel_spmd  # noqa: E402

F32 = mybir.dt.float32
BF = mybir.dt.bfloat16
F8 = mybir.dt.float8e4
I8 = mybir.dt.int8
ALU = mybir.AluOpType
AF = mybir.ActivationFunctionType
DR = mybir.MatmulPerfMode.DoubleRow

B, N, E = 2, 4096, 768
H, HD = 12, 64
NH = 3          # heads per core
M_GROUPS = 4    # head groups (tensor parallel)
GD = NH * HD    # 192 v-dims per core

# Schraudolph exp->fp8e4m3 constants: bits = trunc(s*EXPA + EXPB)
EXPA = 1.4426950408889634   # 8 * log2(e) * 0.125
EXPB = 56.15                # 7*8 + rounding/minimax correction

# exp engine split pattern (by job index): A=ACT exact, D=DVE schraudolph
# (Pool/gpsimd cannot access PSUM, so only ACT and DVE can read scores)
EXP_PATTERN = "ADADADADADADADADADADADAA"  # 13 A, 11 D per 24


def build_nc(n_tokens=N, num_devices=8):
    n = n_tokens
    NQG = n // 512          # 8 q groups of 512
    NKV = n // 128          # 32 kv blocks of 128
    KE = E // 128           # 6 contraction tiles over E

    nc = bacc.Bacc("TRN2", target_bir_lowering=False, debug=False,
                   num_devices=num_devices)

    xT = nc.dram_tensor("xT", [E, n], BF, kind="ExternalInput")
    wqkT = nc.dram_tensor("wqkT", [E, 3 * 128], BF, kind="ExternalInput")
    wvT = nc.dram_tensor("wvT", [E, GD], BF, kind="ExternalInput")
    bq = nc.dram_tensor("bq", [2, 128], F32, kind="ExternalInput")
    wpT = nc.dram_tensor("wpT", [HD, NH, E], BF, kind="ExternalInput")
    out = nc.dram_tensor("out", [n, E], BF, kind="ExternalOutput")

    with tile.TileContext(nc) as tc:
        with (
            tc.tile_pool(name="perm", bufs=1) as perm,
            tc.tile_pool(name="wpool", bufs=1) as wpool,
        ):
            # fp8 Q/K for heads 0 (parts 0:64) and 1 (parts 64:128):
            # [part, q/k, pair-slot, tok]; slot 1 stays zero (DoubleRow pad).
            qk_sb = perm.tile([128, 2, 2, n], F8)
            # head 2 on partitions 0:64
            qk2_sb = perm.tile([64, 2, 2, n], F8)
            # V in [kv-in-block, kv-block, head, hd+ones+pad]: the pad to 80
            # keeps the DoubleRow weight pair stride (3*80=240) 16B-aligned
            # (s3_lw dual-fp8 ISA restriction).
            VP = 80
            v_sb = perm.tile([128, NKV, NH, VP], F8)

            wqkT_sb = wpool.tile([128, KE, 3 * 128], BF)
            wvT_sb = wpool.tile([128, KE, GD], BF)
            wpT_sb = wpool.tile([64, NH, E], BF)
            bq_sb = wpool.tile([128, 2], F32)

            nc.sync.dma_start(wqkT_sb[:], wqkT.rearrange("(a p) c -> p a c", p=128))
            nc.sync.dma_start(wvT_sb[:], wvT.rearrange("(a p) c -> p a c", p=128))
            nc.sync.dma_start(wpT_sb[:], wpT[:])
            nc.sync.dma_start(bq_sb[:], bq.rearrange("a p -> p a"))

            # ones column for denominators (V pad cols are never read)
            nc.gpsimd.memset(v_sb[:, :, :, HD:HD + 1], 1.0)

            with (
                tc.tile_pool(name="bpsum", bufs=1, space="PSUM") as bpsum,
                tc.tile_pool(name="xpool", bufs=2) as xpool,
                tc.tile_pool(name="spool", bufs=3) as spool,
            ):
                # ---- Phase A (KV pass): K and V projections for all
                # tokens. Q is projected lazily (per q-group) under phase B
                # so the exp engines start sooner. xt tiles stay resident.
                xts = {}

                def load_x(ng):
                    qs = slice(ng * 512, (ng + 1) * 512)
                    xt = xpool.tile([128, KE, 512], BF, tag=f"xt{ng}", bufs=1,
                                    name=f"xt{ng}")
                    nc.sync.dma_start(
                        xt[:], xT.rearrange("(a p) t -> p a t", p=128)[:, :, qs])
                    xts[ng] = xt

                def emit_kv(ng):
                    qs = slice(ng * 512, (ng + 1) * 512)
                    xt = xts[ng]
                    nc.gpsimd.memset(qk_sb[:, :, 1, qs].bitcast(I8), 0)
                    nc.gpsimd.memset(qk2_sb[:, :, 1, qs].bitcast(I8), 0)
                    for m in (1, 2):
                        psq = bpsum.tile([128, 512], F32, tag="sc", bufs=3,
                                         name=f"psq{ng}_{m}")
                        for k in range(KE):
                            nc.tensor.matmul(psq[:],
                                             wqkT_sb[:, k, m * 128:(m + 1) * 128],
                                             xt[:, k, :], start=(k == 0),
                                             stop=(k == KE - 1))
                        if m == 1:    # K heads 0,1 -> fp8
                            nc.vector.tensor_copy(qk_sb[:, 1, 0, qs], psq[:])
                        else:         # m2 = [Q2; K2]
                            nc.scalar.activation(qk2_sb[:, 0, 0, qs],
                                                 psq[0:64, :], AF.Identity,
                                                 bias=bq_sb[0:64, 1:2])
                            k2st = xpool.tile([128, 512], F8, tag="k2st",
                                              bufs=2, name=f"k2st{ng}")
                            nc.vector.tensor_copy(k2st[64:128, :],
                                                  psq[64:128, :])
                            nc.sync.dma_start(qk2_sb[:, 1, 0, qs],
                                              k2st[64:128, :])
                    for vj in range(2):
                        psv = bpsum.tile([128, 2, GD], F32, tag="sc", bufs=3,
                                         name=f"psv{ng}_{vj}")
                        for j in range(2):
                            jj = 2 * vj + j
                            for k in range(KE):
                                nc.tensor.matmul(
                                    psv[:, j, :],
                                    xt[:, k, jj * 128:(jj + 1) * 128],
                                    wvT_sb[:, k, :], start=(k == 0),
                                    stop=(k == KE - 1))
                        kv = ng * 4 + 2 * vj
                        src2 = psv.rearrange("p a (h c) -> p a h c", c=HD)
                        nc.vector.tensor_copy(v_sb[:, kv:kv + 2, :, 0:HD],
                                              src2)

                def emit_q(ng):
                    qs = slice(ng * 512, (ng + 1) * 512)
                    psq = bpsum.tile([128, 512], F32, tag="sc", bufs=3,
                                     name=f"psqq{ng}")
                    for k in range(KE):
                        nc.tensor.matmul(psq[:],
                                         wqkT_sb[:, k, 0:128],
                                         xts[ng][:, k, :], start=(k == 0),
                                         stop=(k == KE - 1))
                    nc.scalar.activation(qk_sb[:, 0, 0, qs], psq[:],
                                         AF.Identity, bias=bq_sb[:, 0:1])

                kv_done = [0]

                def need_kv(ng_k):
                    while kv_done[0] <= ng_k:
                        load_x(kv_done[0])
                        emit_kv(kv_done[0])
                        kv_done[0] += 1

                need_kv(0)
                emit_q(0)

                # ---- Phase B: pipelined attention + projection ----
                # scores are emitted DEPTH jobs ahead; norm ops are deferred
                # (recip/bcast at +2, mul at +4, proj at +6) so dependency
                # waits never block the strict-FIFO DVE/ACT queues.
                def qk_aps(h, kv, qg):
                    qs = slice(qg * 512, (qg + 1) * 512)
                    ks = slice(kv * 128, (kv + 1) * 128)
                    if h == 2:
                        return qk2_sb[:, 1, :, ks], qk2_sb[:, 0, :, qs]
                    pb = 64 * h
                    return (qk_sb[pb:pb + 64, 1, :, ks],
                            qk_sb[pb:pb + 64, 0, :, qs])

                DEPTH = 3
                # (h0,h1) paired kp-waves first (2 concurrent PV accumulators
                # = the 2 "acc" buffers), then the h2 run. During startup the
                # paired waves consume kv blocks at the rate the lazy KV pass
                # produces them, keeping the exp engines fed.
                jobs = []
                for qg in range(NQG):
                    for kp in range(NKV // 2):
                        jobs += [(qg, 0, kp), (qg, 1, kp)]
                    jobs += [(qg, 2, kp) for kp in range(NKV // 2)]
                ei = 0
                pv_tiles = {}
                yn_tiles = {}
                todo = []   # (trigger_idx, fn)

                def emit_scores(qg, h, kp):
                    need_kv((2 * kp + 1) // 4)
                    sc = bpsum.tile([128, 2, 512], F32, tag="sc", bufs=3,
                                    name=f"sc{qg}_{h}_{kp}")
                    for jj in range(2):
                        lhsT, rhs = qk_aps(h, 2 * kp + jj, qg)
                        nc.tensor.matmul(sc[:, jj, :], lhsT, rhs,
                                         start=True, stop=True, perf_mode=DR)
                    return sc

                def emit_exp(sc, qg, h, kp):
                    nonlocal ei
                    pt = spool.tile([128, 2, 512], F8, tag="p", bufs=6,
                                    name=f"p{qg}_{h}_{kp}")
                    e = EXP_PATTERN[ei % len(EXP_PATTERN)]
                    ei += 1
                    if e == "A":
                        nc.scalar.activation(pt[:], sc[:], AF.Exp, scale=0.125)
                    else:
                        nc.vector.tensor_scalar(pt.bitcast(I8), sc[:],
                                                EXPA, EXPB, ALU.mult, ALU.add)
                    return pt

                def emit_recip_bcast(qg, h):
                    pv = pv_tiles[(qg, h)]
                    rcp = spool.tile([1, 512], F32, tag="rcp", bufs=2,
                                     name=f"rcp{qg}_{h}")
                    nc.vector.reciprocal(rcp[:], pv[64:65, :])
                    rb = spool.tile([64, 512], F32, tag="rb", bufs=2,
                                    name=f"rb{qg}_{h}")
                    nc.gpsimd.partition_broadcast(rb[:], rcp[:])
                    return rb

                def emit_mul(qg, h, rb):
                    pv = pv_tiles[(qg, h)]
                    yn = spool.tile([64, 512], BF, tag="yn", bufs=6,
                                    name=f"yn{qg}_{h}")
                    nc.vector.tensor_tensor(yn[:], pv[0:64, :], rb[:],
                                            ALU.mult)
                    yn_tiles[(qg, h)] = yn

                def emit_proj(qg):
                    for f in range(2):
                        fw = 512 if f == 0 else E - 512
                        fsl = slice(f * 512, f * 512 + fw)
                        for qb in range(4):
                            pp = bpsum.tile([128, fw], F32, tag="acc",
                                            bufs=2, name=f"pp{qg}_{f}_{qb}")
                            for h in range(NH):
                                nc.tensor.matmul(
                                    pp[:],
                                    yn_tiles[(qg, h)][:, qb * 128:(qb + 1) * 128],
                                    wpT_sb[:, h, fsl],
                                    start=(h == 0), stop=(h == NH - 1))
                            ost = spool.tile([128, fw], BF, tag="ost", bufs=4,
                                             name=f"ost{qg}_{f}_{qb}")
                            nc.scalar.copy(ost[:], pp[:])
                            nc.sync.dma_start(
                                out[qg * 512 + qb * 128:
                                    qg * 512 + (qb + 1) * 128, fsl], ost[:])
                    if qg + 2 < NQG:
                        emit_q(qg + 2)

                def norm_closure(qg, h, idx):
                    d1, d2 = (1, 3) if h < 2 else (2, 4)
                    def stage1():
                        rb = emit_recip_bcast(qg, h)
                        todo.append((idx + d2, lambda: emit_mul(qg, h, rb)))
                    todo.append((idx + d1, stage1))
                    if h == NH - 1:
                        todo.append((idx + 6, lambda: emit_proj(qg)))

                todo.append((32, lambda: emit_q(1)))
                pending = [emit_scores(*jobs[i]) for i in range(DEPTH)]
                for idx, (qg, h, kp) in enumerate(jobs):
                    sc = pending.pop(0)
                    pt = emit_exp(sc, qg, h, kp)
                    if idx + DEPTH < len(jobs):
                        pending.append(emit_scores(*jobs[idx + DEPTH]))
                    if kp == 0:
                        pv = bpsum.tile([65, 512], F32, tag="acc", bufs=2,
                                        name=f"pv{qg}_{h}")
                        pv_tiles[(qg, h)] = pv[:]
                    nc.tensor.matmul(pv_tiles[(qg, h)],
                                     v_sb[:, 2 * kp:2 * kp + 2, h, 0:HD + 1],
                                     pt[:], start=(kp == 0),
                                     stop=(kp == NKV // 2 - 1), perf_mode=DR)
                    todo.sort(key=lambda t: t[0])
                    while todo and idx >= todo[0][0]:
                        todo.pop(0)[1]()
                    if kp == NKV // 2 - 1:
                        norm_closure(qg, h, idx)
                while todo:
                    todo.sort(key=lambda t: t[0])
                    todo.pop(0)[1]()

    nc.finalize()
    return nc


def host_prep(x, w_qkv, b_qkv, w_proj, b_proj, n_tokens=N):
    """Build per-core input maps + the host-side combine closure."""
    x = np.asarray(x, np.float32)
    w_qkv = np.asarray(w_qkv, np.float32)
    b_qkv = np.asarray(b_qkv, np.float32)
    w_proj = np.asarray(w_proj, np.float32)
    b_proj = np.asarray(b_proj, np.float32)

    xT = [np.ascontiguousarray(x[b].T).astype(ml_dtypes.bfloat16)
          for b in range(B)]

    in_maps = []
    for c in range(8):
        b, g = divmod(c, M_GROUPS)
        base = g * NH * 3 * HD  # row offset of this group in w_qkv
        wq = [w_qkv[base + i * 3 * HD: base + i * 3 * HD + HD]
              for i in range(NH)]
        wk = [w_qkv[base + i * 3 * HD + HD: base + i * 3 * HD + 2 * HD]
              for i in range(NH)]
        wv = [w_qkv[base + i * 3 * HD + 2 * HD: base + i * 3 * HD + 3 * HD]
              for i in range(NH)]
        bqv = [b_qkv[base + i * 3 * HD: base + i * 3 * HD + HD]
               for i in range(NH)]
        # m-tiles: m0=[Q0;Q1], m1=[K0;K1], m2=[Q2;K2]
        wqkT = np.concatenate(
            [wq[0], wq[1], wk[0], wk[1], wq[2], wk[2]], axis=0).T  # [E, 384]
        wvT = np.concatenate(wv, axis=0).T  # [E, 192]
        bqa = np.zeros((2, 128), np.float32)
        bqa[0, 0:HD] = bqv[0]
        bqa[0, HD:2 * HD] = bqv[1]
        bqa[1, 0:HD] = bqv[2]
        wp = w_proj[:, g * GD:(g + 1) * GD]  # [768, 192]
        wpT = np.ascontiguousarray(
            wp.T.reshape(NH, HD, E).transpose(1, 0, 2))  # [64, 3, 768]
        in_maps.append({
            "xT": np.ascontiguousarray(xT[b]),
            "wqkT": np.ascontiguousarray(wqkT).astype(ml_dtypes.bfloat16),
            "wvT": np.ascontiguousarray(wvT).astype(ml_dtypes.bfloat16),
            "bq": bqa,
            "wpT": wpT.astype(ml_dtypes.bfloat16),
        })

    bv_all = np.concatenate(
        [b_qkv[h * 3 * HD + 2 * HD: (h + 1) * 3 * HD] for h in range(H)])
    b_eff = b_proj + w_proj @ bv_all

    def combine(results):
        o = np.empty((B, n_tokens, E), np.float32)
        for b in range(B):
            acc = results[b * M_GROUPS]["out"].astype(np.float32)
            for g in range(1, M_GROUPS):
                acc = acc + results[b * M_GROUPS + g]["out"].astype(np.float32)
            o[b] = acc + b_eff
        return o

    return in_maps, combine


_NC_CACHE = {}


def kernel(x, w_qkv, b_qkv, w_proj, b_proj):
    if "nc" not in _NC_CACHE:
        _NC_CACHE["nc"] = build_nc()
    nc = _NC_CACHE["nc"]
    in_maps, combine = host_prep(x, w_qkv, b_qkv, w_proj, b_proj)
    res = run_bass_kernel_spmd(nc, in_maps, core_ids=list(range(8)))
    return combine(res.results)


if __name__ == "__main__":
    rng = np.random.default_rng(0)
    inputs = {
        "x": rng.normal(size=(B, N, E)).astype(np.float32),
        "w_qkv": (rng.normal(size=(3 * E, E)) * 0.02).astype(np.float32),
        "b_qkv": (rng.normal(size=(3 * E,)) * 0.02).astype(np.float32),
        "w_proj": (rng.normal(size=(E, E)) * 0.02).astype(np.float32),
        "b_proj": (rng.normal(size=(E,)) * 0.02).astype(np.float32),
    }
    o = kernel(**inputs)
    print("out", o.shape, o.dtype, float(np.abs(o).mean()))


# revision 17
# speedup vs baseline: 1.0769x; 1.0769x over previous
"""Multi-head attention Bass kernel for Trainium2 (8 NeuronCores).

Problem: B=2, N=4096, E=768, H=12 heads of dim 64 (nn_MultiHeadAttention).
Sharding: 2 batches x 4 head-groups (3 heads each) = 8 cores.

v2 design (mixed precision, three-engine softmax):
  - QKV projection in bf16 (x, w_qkv bf16; fp32 PSUM accumulation).
  - Q/K/V stored fp8e4m3 in SBUF; scores and P@V matmuls run in fp8
    DoubleRow perf mode (0.5 cycles/row): scores use a zeroed second
    K-slot pair, P@V pairs two kv-blocks per instruction.
  - exp(s/8) is computed by THREE engines in parallel: ACT does exact
    Exp -> fp8 writes; DVE and Pool (gpsimd) use a one-pass Schraudolph:
    int8 bits = trunc(s*log2(e) + 56.15) reinterpreted as fp8e4m3.
  - softmax denominators via a ones-column appended to V (65th row of
    the P@V output); normalization = partition_broadcast + divide.
  - output projection in bf16 against w_proj rows; partial [N, E]
    written as bf16; host sums the 4 partials and adds bias.

Bias handling (exact algebra): K bias drops out of softmax; V bias is
folded into b_proj on the host; Q bias added on-device (fp32) before
the fp8 quantization of Q.
"""

import sys

sys.path.insert(0, "/opt/trn_rl_repo")

import numpy as np
import ml_dtypes

import concourse.bass as bass  # noqa: E402
import concourse.mybir as mybir  # noqa: E402
import concourse.tile as tile  # noqa: E402
from concourse import bacc  # noqa: E402
from concourse.bass_utils import run_bass_kernel_spmd  # noqa: E402

F32 = mybir.dt.float32
BF = mybir.dt.bfloat16
F8 = mybir.dt.float8e4
I8 = mybir.dt.int8
ALU = mybir.AluOpType
AF = mybir.ActivationFunctionType
DR = mybir.MatmulPerfMode.DoubleRow

B, N, E = 2, 4096, 768
H, HD = 12, 64
NH = 3          # heads per core
M_GROUPS = 4    # head groups (tensor parallel)
GD = NH * HD    # 192 v-dims per core

# Schraudolph exp->fp8e4m3 constants: bits = trunc(s*EXPA + EXPB)
EXPA = 1.4426950408889634   # 8 * log2(e) * 0.125
EXPB = 56.15                # 7*8 + rounding/minimax correction

# exp engine split pattern (by job index): A=ACT exact, D=DVE schraudolph
# (Pool/gpsimd cannot access PSUM, so only ACT and DVE can read scores)
EXP_PATTERN = "ADADADADADADADADADADADAA"  # 13 A, 11 D per 24


def build_nc(n_tokens=N, num_devices=8):
    n = n_tokens
    NQG = n // 512          # 8 q groups of 512
    NKV = n // 128          # 32 kv blocks of 128
    KE = E // 128           # 6 contraction tiles over E

    nc = bacc.Bacc("TRN2", target_bir_lowering=False, debug=False,
                   num_devices=num_devices)

    xT = nc.dram_tensor("xT", [E, n], BF, kind="ExternalInput")
    wqkT = nc.dram_tensor("wqkT", [E, 3 * 128], BF, kind="ExternalInput")
    wvT = nc.dram_tensor("wvT", [E, GD], BF, kind="ExternalInput")
    bq = nc.dram_tensor("bq", [2, 128], F32, kind="ExternalInput")
    wpT = nc.dram_tensor("wpT", [HD, NH, E], BF, kind="ExternalInput")
    out = nc.dram_tensor("out", [n, E], BF, kind="ExternalOutput")

    with tile.TileContext(nc) as tc:
        with (
            tc.tile_pool(name="perm", bufs=1) as perm,
            tc.tile_pool(name="wpool", bufs=1) as wpool,
        ):
            # fp8 Q/K for heads 0 (parts 0:64) and 1 (parts 64:128):
            # [part, q/k, pair-slot, tok]; slot 1 stays zero (DoubleRow pad).
            qk_sb = perm.tile([128, 2, 2, n], F8)
            # head 2 on partitions 0:64
            qk2_sb = perm.tile([64, 2, 2, n], F8)
            # V in [kv-in-block, kv-block, head, hd+ones+pad]: the pad to 80
            # keeps the DoubleRow weight pair stride (3*80=240) 16B-aligned
            # (s3_lw dual-fp8 ISA restriction).
            VP = 80
            v_sb = perm.tile([128, NKV, NH, VP], F8)

            wqkT_sb = wpool.tile([128, KE, 3 * 128], BF)
            wvT_sb = wpool.tile([128, KE, GD], BF)
            wpT_sb = wpool.tile([64, NH, E], BF)
            bq_sb = wpool.tile([128, 2], F32)

            nc.sync.dma_start(wqkT_sb[:], wqkT.rearrange("(a p) c -> p a c", p=128))
            nc.sync.dma_start(wvT_sb[:], wvT.rearrange("(a p) c -> p a c", p=128))
            nc.sync.dma_start(wpT_sb[:], wpT[:])
            nc.sync.dma_start(bq_sb[:], bq.rearrange("a p -> p a"))

            # ones column for denominators (V pad cols are never read)
            nc.gpsimd.memset(v_sb[:, :, :, HD:HD + 1], 1.0)

            with (
                tc.tile_pool(name="bpsum", bufs=1, space="PSUM") as bpsum,
                tc.tile_pool(name="xpool", bufs=2) as xpool,
                tc.tile_pool(name="spool", bufs=3) as spool,
            ):
                # ---- Phase A (KV pass): K and V projections for all
                # tokens. Q is projected lazily (per q-group) under phase B
                # so the exp engines start sooner. xt tiles stay resident.
                xts = {}

                def load_x(ng):
                    qs = slice(ng * 512, (ng + 1) * 512)
                    xt = xpool.tile([128, KE, 512], BF, tag=f"xt{ng}", bufs=1,
                                    name=f"xt{ng}")
                    nc.sync.dma_start(
                        xt[:], xT.rearrange("(a p) t -> p a t", p=128)[:, :, qs])
                    xts[ng] = xt

                def emit_kv(ng):
                    qs = slice(ng * 512, (ng + 1) * 512)
                    xt = xts[ng]
                    nc.gpsimd.memset(qk_sb[:, :, 1, qs].bitcast(I8), 0)
                    nc.gpsimd.memset(qk2_sb[:, :, 1, qs].bitcast(I8), 0)
                    for m in (1, 2):
                        psq = bpsum.tile([128, 512], F32, tag="sc", bufs=3,
                                         name=f"psq{ng}_{m}")
                        for k in range(KE):
                            nc.tensor.matmul(psq[:],
                                             wqkT_sb[:, k, m * 128:(m + 1) * 128],
                                             xt[:, k, :], start=(k == 0),
                                             stop=(k == KE - 1))
                        if m == 1:    # K heads 0,1 -> fp8
                            nc.vector.tensor_copy(qk_sb[:, 1, 0, qs], psq[:])
                        else:         # m2 = [Q2; K2]
                            nc.scalar.activation(qk2_sb[:, 0, 0, qs],
                                                 psq[0:64, :], AF.Identity,
                                                 bias=bq_sb[0:64, 1:2])
                            k2st = xpool.tile([128, 512], F8, tag="k2st",
                                              bufs=2, name=f"k2st{ng}")
                            nc.vector.tensor_copy(k2st[64:128, :],
                                                  psq[64:128, :])
                            nc.sync.dma_start(qk2_sb[:, 1, 0, qs],
                                              k2st[64:128, :])
                    for vj in range(2):
                        psv = bpsum.tile([128, 2, GD], F32, tag="sc", bufs=3,
                                         name=f"psv{ng}_{vj}")
                        for j in range(2):
                            jj = 2 * vj + j
                            for k in range(KE):
                                nc.tensor.matmul(
                                    psv[:, j, :],
                                    xt[:, k, jj * 128:(jj + 1) * 128],
                                    wvT_sb[:, k, :], start=(k == 0),
                                    stop=(k == KE - 1))
                        kv = ng * 4 + 2 * vj
                        src2 = psv.rearrange("p a (h c) -> p a h c", c=HD)
                        nc.vector.tensor_copy(v_sb[:, kv:kv + 2, :, 0:HD],
                                              src2)

                def emit_q(ng):
                    qs = slice(ng * 512, (ng + 1) * 512)
                    psq = bpsum.tile([128, 512], F32, tag="sc", bufs=3,
                                     name=f"psqq{ng}")
                    for k in range(KE):
                        nc.tensor.matmul(psq[:],
                                         wqkT_sb[:, k, 0:128],
                                         xts[ng][:, k, :], start=(k == 0),
                                         stop=(k == KE - 1))
                    nc.scalar.activation(qk_sb[:, 0, 0, qs], psq[:],
                                         AF.Identity, bias=bq_sb[:, 0:1])

                kv_done = [0]

                def need_kv(ng_k):
                    while kv_done[0] <= ng_k:
                        load_x(kv_done[0])
                        emit_kv(kv_done[0])
                        kv_done[0] += 1

                need_kv(0)
                emit_q(0)

                # ---- Phase B: pipelined attention + projection ----
                # scores are emitted DEPTH jobs ahead; norm ops are deferred
                # (recip/bcast at +2, mul at +4, proj at +6) so dependency
                # waits never block the strict-FIFO DVE/ACT queues.
                def qk_aps(h, kv, qg):
                    qs = slice(qg * 512, (qg + 1) * 512)
                    ks = slice(kv * 128, (kv + 1) * 128)
                    if h == 2:
                        return qk2_sb[:, 1, :, ks], qk2_sb[:, 0, :, qs]
                    pb = 64 * h
                    return (qk_sb[pb:pb + 64, 1, :, ks],
                            qk_sb[pb:pb + 64, 0, :, qs])

                DEPTH = 3
                # (h0,h1) paired kp-waves first (2 concurrent PV accumulators
                # = the 2 "acc" buffers), then the h2 run. During startup the
                # paired waves consume kv blocks at the rate the lazy KV pass
                # produces them, keeping the exp engines fed.
                jobs = []
                for kp in range(NKV // 2):      # qg0: paired waves (startup)
                    jobs += [(0, 0, kp), (0, 1, kp)]
                jobs += [(0, 2, kp) for kp in range(NKV // 2)]
                for qg in range(1, NQG):        # steady state: heads in turn
                    jobs += [(qg, h, kp) for h in range(NH)
                             for kp in range(NKV // 2)]
                ei = 0
                pv_tiles = {}
                yn_tiles = {}
                todo = []   # (trigger_idx, fn)

                def emit_scores(qg, h, kp):
                    need_kv((2 * kp + 1) // 4)
                    sc = bpsum.tile([128, 2, 512], F32, tag="sc", bufs=3,
                                    name=f"sc{qg}_{h}_{kp}")
                    for jj in range(2):
                        lhsT, rhs = qk_aps(h, 2 * kp + jj, qg)
                        nc.tensor.matmul(sc[:, jj, :], lhsT, rhs,
                                         start=True, stop=True, perf_mode=DR)
                    return sc

                def emit_exp(sc, qg, h, kp):
                    nonlocal ei
                    pt = spool.tile([128, 2, 512], F8, tag="p", bufs=6,
                                    name=f"p{qg}_{h}_{kp}")
                    e = EXP_PATTERN[ei % len(EXP_PATTERN)]
                    ei += 1
                    if e == "A":
                        nc.scalar.activation(pt[:], sc[:], AF.Exp, scale=0.125)
                    else:
                        nc.vector.tensor_scalar(pt.bitcast(I8), sc[:],
                                                EXPA, EXPB, ALU.mult, ALU.add)
                    return pt

                def emit_recip_bcast(qg, h):
                    pv = pv_tiles[(qg, h)]
                    rcp = spool.tile([1, 512], F32, tag="rcp", bufs=2,
                                     name=f"rcp{qg}_{h}")
                    nc.vector.reciprocal(rcp[:], pv[64:65, :])
                    rb = spool.tile([64, 512], F32, tag="rb", bufs=2,
                                    name=f"rb{qg}_{h}")
                    nc.gpsimd.partition_broadcast(rb[:], rcp[:])
                    return rb

                def emit_mul(qg, h, rb):
                    pv = pv_tiles[(qg, h)]
                    yn = spool.tile([64, 512], BF, tag="yn", bufs=6,
                                    name=f"yn{qg}_{h}")
                    nc.vector.tensor_tensor(yn[:], pv[0:64, :], rb[:],
                                            ALU.mult)
                    yn_tiles[(qg, h)] = yn

                def emit_proj(qg):
                    for f in range(2):
                        fw = 512 if f == 0 else E - 512
                        fsl = slice(f * 512, f * 512 + fw)
                        for qb in range(4):
                            pp = bpsum.tile([128, fw], F32, tag="acc",
                                            bufs=2, name=f"pp{qg}_{f}_{qb}")
                            for h in range(NH):
                                nc.tensor.matmul(
                                    pp[:],
                                    yn_tiles[(qg, h)][:, qb * 128:(qb + 1) * 128],
                                    wpT_sb[:, h, fsl],
                                    start=(h == 0), stop=(h == NH - 1))
                            ost = spool.tile([128, fw], BF, tag="ost", bufs=4,
                                             name=f"ost{qg}_{f}_{qb}")
                            nc.scalar.copy(ost[:], pp[:])
                            nc.sync.dma_start(
                                out[qg * 512 + qb * 128:
                                    qg * 512 + (qb + 1) * 128, fsl], ost[:])
                    if qg + 2 < NQG:
                        emit_q(qg + 2)

                def norm_closure(qg, h, idx):
                    d1, d2 = (1, 3) if h < 2 else (2, 4)
                    def stage1():
                        rb = emit_recip_bcast(qg, h)
                        todo.append((idx + d2, lambda: emit_mul(qg, h, rb)))
                    todo.append((idx + d1, stage1))
                    if h == NH - 1:
                        todo.append((idx + 6, lambda: emit_proj(qg)))

                todo.append((32, lambda: emit_q(1)))
                pending = [emit_scores(*jobs[i]) for i in range(DEPTH)]
                for idx, (qg, h, kp) in enumerate(jobs):
                    sc = pending.pop(0)
                    pt = emit_exp(sc, qg, h, kp)
                    if idx + DEPTH < len(jobs):
                        pending.append(emit_scores(*jobs[idx + DEPTH]))
                    if kp == 0:
                        pv = bpsum.tile([65, 512], F32, tag="acc", bufs=2,
                                        name=f"pv{qg}_{h}")
                        pv_tiles[(qg, h)] = pv[:]
                    nc.tensor.matmul(pv_tiles[(qg, h)],
                                     v_sb[:, 2 * kp:2 * kp + 2, h, 0:HD + 1],
                                     pt[:], start=(kp == 0),
                                     stop=(kp == NKV // 2 - 1), perf_mode=DR)
                    todo.sort(key=lambda t: t[0])
                    while todo and idx >= todo[0][0]:
                        todo.pop(0)[1]()
                    if kp == NKV // 2 - 1:
                        norm_closure(qg, h, idx)
                while todo:
                    todo.sort(key=lambda t: t[0])
                    todo.pop(0)[1]()

    nc.finalize()
    return nc


def host_prep(x, w_qkv, b_qkv, w_proj, b_proj, n_tokens=N):
    """Build per-core input maps + the host-side combine closure."""
    x = np.asarray(x, np.float32)
    w_qkv = np.asarray(w_qkv, np.float32)
    b_qkv = np.asarray(b_qkv, np.float32)
    w_proj = np.asarray(w_proj, np.float32)
    b_proj = np.asarray(b_proj, np.float32)

    xT = [np.ascontiguousarray(x[b].T).astype(ml_dtypes.bfloat16)
          for b in range(B)]

    in_maps = []
    for c in range(8):
        b, g = divmod(c, M_GROUPS)
        base = g * NH * 3 * HD  # row offset of this group in w_qkv
        wq = [w_qkv[base + i * 3 * HD: base + i * 3 * HD + HD]
              for i in range(NH)]
        wk = [w_qkv[base + i * 3 * HD + HD: base + i * 3 * HD + 2 * HD]
              for i in range(NH)]
        wv = [w_qkv[base + i * 3 * HD + 2 * HD: base + i * 3 * HD + 3 * HD]
              for i in range(NH)]
        bqv = [b_qkv[base + i * 3 * HD: base + i * 3 * HD + HD]
               for i in range(NH)]
        # m-tiles: m0=[Q0;Q1], m1=[K0;K1], m2=[Q2;K2]
        wqkT = np.concatenate(
            [wq[0], wq[1], wk[0], wk[1], wq[2], wk[2]], axis=0).T  # [E, 384]
        wvT = np.concatenate(wv, axis=0).T  # [E, 192]
        bqa = np.zeros((2, 128), np.float32)
        bqa[0, 0:HD] = bqv[0]
        bqa[0, HD:2 * HD] = bqv[1]
        bqa[1, 0:HD] = bqv[2]
        wp = w_proj[:, g * GD:(g + 1) * GD]  # [768, 192]
        wpT = np.ascontiguousarray(
            wp.T.reshape(NH, HD, E).transpose(1, 0, 2))  # [64, 3, 768]
        in_maps.append({
            "xT": np.ascontiguousarray(xT[b]),
            "wqkT": np.ascontiguousarray(wqkT).astype(ml_dtypes.bfloat16),
            "wvT": np.ascontiguousarray(wvT).astype(ml_dtypes.bfloat16),
            "bq": bqa,
            "wpT": wpT.astype(ml_dtypes.bfloat16),
        })

    bv_all = np.concatenate(
        [b_qkv[h * 3 * HD + 2 * HD: (h + 1) * 3 * HD] for h in range(H)])
    b_eff = b_proj + w_proj @ bv_all

    def combine(results):
        o = np.empty((B, n_tokens, E), np.float32)
        for b in range(B):
            acc = results[b * M_GROUPS]["out"].astype(np.float32)
            for g in range(1, M_GROUPS):
                acc = acc + results[b * M_GROUPS + g]["out"].astype(np.float32)
            o[b] = acc + b_eff
        return o

    return in_maps, combine


_NC_CACHE = {}


def kernel(x, w_qkv, b_qkv, w_proj, b_proj):
    if "nc" not in _NC_CACHE:
        _NC_CACHE["nc"] = build_nc()
    nc = _NC_CACHE["nc"]
    in_maps, combine = host_prep(x, w_qkv, b_qkv, w_proj, b_proj)
    res = run_bass_kernel_spmd(nc, in_maps, core_ids=list(range(8)))
    return combine(res.results)


if __name__ == "__main__":
    rng = np.random.default_rng(0)
    inputs = {
        "x": rng.normal(size=(B, N, E)).astype(np.float32),
        "w_qkv": (rng.normal(size=(3 * E, E)) * 0.02).astype(np.float32),
        "b_qkv": (rng.normal(size=(3 * E,)) * 0.02).astype(np.float32),
        "w_proj": (rng.normal(size=(E, E)) * 0.02).astype(np.float32),
        "b_proj": (rng.normal(size=(E,)) * 0.02).astype(np.float32),
    }
    o = kernel(**inputs)
    print("out", o.shape, o.dtype, float(np.abs(o).mean()))


# revision 20
# speedup vs baseline: 1.0813x; 1.0041x over previous
"""Multi-head attention Bass kernel for Trainium2 (8 NeuronCores).

Problem: B=2, N=4096, E=768, H=12 heads of dim 64 (nn_MultiHeadAttention).
Sharding: 2 batches x 4 head-groups (3 heads each) = 8 cores.

v2 design (mixed precision, three-engine softmax):
  - QKV projection in bf16 (x, w_qkv bf16; fp32 PSUM accumulation).
  - Q/K/V stored fp8e4m3 in SBUF; scores and P@V matmuls run in fp8
    DoubleRow perf mode (0.5 cycles/row): scores use a zeroed second
    K-slot pair, P@V pairs two kv-blocks per instruction.
  - exp(s/8) is computed by THREE engines in parallel: ACT does exact
    Exp -> fp8 writes; DVE and Pool (gpsimd) use a one-pass Schraudolph:
    int8 bits = trunc(s*log2(e) + 56.15) reinterpreted as fp8e4m3.
  - softmax denominators via a ones-column appended to V (65th row of
    the P@V output); normalization = partition_broadcast + divide.
  - output projection in bf16 against w_proj rows; partial [N, E]
    written as bf16; host sums the 4 partials and adds bias.

Bias handling (exact algebra): K bias drops out of softmax; V bias is
folded into b_proj on the host; Q bias added on-device (fp32) before
the fp8 quantization of Q.
"""

import sys

sys.path.insert(0, "/opt/trn_rl_repo")

import numpy as np
import ml_dtypes

import concourse.bass as bass  # noqa: E402
import concourse.mybir as mybir  # noqa: E402
import concourse.tile as tile  # noqa: E402
from concourse import bacc  # noqa: E402
from concourse.bass_utils import run_bass_kernel_spmd  # noqa: E402

F32 = mybir.dt.float32
BF = mybir.dt.bfloat16
F8 = mybir.dt.float8e4
I8 = mybir.dt.int8
ALU = mybir.AluOpType
AF = mybir.ActivationFunctionType
DR = mybir.MatmulPerfMode.DoubleRow

B, N, E = 2, 4096, 768
H, HD = 12, 64
NH = 3          # heads per core
M_GROUPS = 4    # head groups (tensor parallel)
GD = NH * HD    # 192 v-dims per core

# Schraudolph exp->fp8e4m3 constants: bits = trunc(s*EXPA + EXPB)
EXPA = 1.4426950408889634   # 8 * log2(e) * 0.125
EXPB = 56.15                # 7*8 + rounding/minimax correction

# exp engine split pattern (by job index): A=ACT exact, D=DVE schraudolph
# (Pool/gpsimd cannot access PSUM, so only ACT and DVE can read scores)
EXP_PATTERN = "ADADADADADADADADADADADAA"  # 13 A, 11 D per 24


def build_nc(n_tokens=N, num_devices=8):
    n = n_tokens
    NQG = n // 512          # 8 q groups of 512
    NKV = n // 128          # 32 kv blocks of 128
    KE = E // 128           # 6 contraction tiles over E

    nc = bacc.Bacc("TRN2", target_bir_lowering=False, debug=False,
                   num_devices=num_devices)

    xT = nc.dram_tensor("xT", [E, n], BF, kind="ExternalInput")
    wqkT = nc.dram_tensor("wqkT", [E, 3 * 128], BF, kind="ExternalInput")
    wvT = nc.dram_tensor("wvT", [E, GD], BF, kind="ExternalInput")
    bq = nc.dram_tensor("bq", [2, 128], F32, kind="ExternalInput")
    wpT = nc.dram_tensor("wpT", [HD, NH, E], BF, kind="ExternalInput")
    out = nc.dram_tensor("out", [n, E], BF, kind="ExternalOutput")

    with tile.TileContext(nc) as tc:
        with (
            tc.tile_pool(name="perm", bufs=1) as perm,
            tc.tile_pool(name="wpool", bufs=1) as wpool,
        ):
            # fp8 Q/K for heads 0 (parts 0:64) and 1 (parts 64:128):
            # [part, q/k, pair-slot, tok]; slot 1 stays zero (DoubleRow pad).
            qk_sb = perm.tile([128, 2, 2, n], F8)
            # head 2 on partitions 0:64
            qk2_sb = perm.tile([64, 2, 2, n], F8)
            # V in [kv-in-block, kv-block, head, hd+ones+pad]: the pad to 80
            # keeps the DoubleRow weight pair stride (3*80=240) 16B-aligned
            # (s3_lw dual-fp8 ISA restriction).
            VP = 80
            v_sb = perm.tile([128, NKV, NH, VP], F8)

            wqkT_sb = wpool.tile([128, KE, 3 * 128], BF)
            wvT_sb = wpool.tile([128, KE, GD], BF)
            wpT_sb = wpool.tile([64, NH, E], BF)
            bq_sb = wpool.tile([128, 2], F32)

            nc.sync.dma_start(wqkT_sb[:], wqkT.rearrange("(a p) c -> p a c", p=128))
            nc.sync.dma_start(wvT_sb[:], wvT.rearrange("(a p) c -> p a c", p=128))
            nc.sync.dma_start(wpT_sb[:], wpT[:])
            nc.sync.dma_start(bq_sb[:], bq.rearrange("a p -> p a"))

            # ones column for denominators (V pad cols are never read)
            nc.gpsimd.memset(v_sb[:, :, :, HD:HD + 1], 1.0)

            with (
                tc.tile_pool(name="bpsum", bufs=1, space="PSUM") as bpsum,
                tc.tile_pool(name="xpool", bufs=2) as xpool,
                tc.tile_pool(name="spool", bufs=3) as spool,
            ):
                # ---- Phase A (KV pass): K and V projections for all
                # tokens. Q is projected lazily (per q-group) under phase B
                # so the exp engines start sooner. xt tiles stay resident.
                xts = {}

                def load_x(ng):
                    qs = slice(ng * 512, (ng + 1) * 512)
                    xt = xpool.tile([128, KE, 512], BF, tag=f"xt{ng}", bufs=1,
                                    name=f"xt{ng}")
                    nc.sync.dma_start(
                        xt[:], xT.rearrange("(a p) t -> p a t", p=128)[:, :, qs])
                    xts[ng] = xt

                def kv_chains(ng):
                    qs = slice(ng * 512, (ng + 1) * 512)

                    def chain_m(m):
                        def _f():
                            xt = xts[ng]
                            nc.gpsimd.memset(
                                qk_sb[:, m - 1, 1, qs].bitcast(I8), 0)
                            nc.gpsimd.memset(
                                qk2_sb[:, m - 1, 1, qs].bitcast(I8), 0)
                            psq = bpsum.tile([128, 512], F32, tag="sc",
                                             bufs=3, name=f"psq{ng}_{m}")
                            for k in range(KE):
                                nc.tensor.matmul(
                                    psq[:],
                                    wqkT_sb[:, k, m * 128:(m + 1) * 128],
                                    xt[:, k, :], start=(k == 0),
                                    stop=(k == KE - 1))
                            if m == 1:    # K heads 0,1 -> fp8
                                nc.vector.tensor_copy(qk_sb[:, 1, 0, qs],
                                                      psq[:])
                            else:         # m2 = [Q2; K2]
                                nc.scalar.activation(qk2_sb[:, 0, 0, qs],
                                                     psq[0:64, :], AF.Identity,
                                                     bias=bq_sb[0:64, 1:2])
                                k2st = xpool.tile([128, 512], F8, tag="k2st",
                                                  bufs=2, name=f"k2st{ng}")
                                nc.vector.tensor_copy(k2st[64:128, :],
                                                      psq[64:128, :])
                                nc.sync.dma_start(qk2_sb[:, 1, 0, qs],
                                                  k2st[64:128, :])
                        return _f

                    def chain_v(vj):
                        def _f():
                            xt = xts[ng]
                            psv = bpsum.tile([128, 2, GD], F32, tag="sc",
                                             bufs=3, name=f"psv{ng}_{vj}")
                            for j in range(2):
                                jj = 2 * vj + j
                                for k in range(KE):
                                    nc.tensor.matmul(
                                        psv[:, j, :],
                                        xt[:, k, jj * 128:(jj + 1) * 128],
                                        wvT_sb[:, k, :], start=(k == 0),
                                        stop=(k == KE - 1))
                            kv = ng * 4 + 2 * vj
                            src2 = psv.rearrange("p a (h c) -> p a h c", c=HD)
                            nc.vector.tensor_copy(
                                v_sb[:, kv:kv + 2, :, 0:HD], src2)
                        return _f
                    return [chain_m(1), chain_m(2), chain_v(0), chain_v(1)]

                def qk_aps(h, kv, qg):
                    qs = slice(qg * 512, (qg + 1) * 512)
                    ks = slice(kv * 128, (kv + 1) * 128)
                    if h == 2:
                        return qk2_sb[:, 1, :, ks], qk2_sb[:, 0, :, qs]
                    pb = 64 * h
                    return (qk_sb[pb:pb + 64, 1, :, ks],
                            qk_sb[pb:pb + 64, 0, :, qs])

                DEPTH = 3
                # (h0,h1) paired kp-waves first (2 concurrent PV accumulators
                # = the 2 "acc" buffers), then the h2 run. During startup the
                # paired waves consume kv blocks at the rate the lazy KV pass
                # produces them, keeping the exp engines fed.
                jobs = []
                for kp in range(NKV // 2):      # qg0: paired waves (startup)
                    jobs += [(0, 0, kp), (0, 1, kp)]
                jobs += [(0, 2, kp) for kp in range(NKV // 2)]
                for qg in range(1, NQG):        # steady state: heads in turn
                    jobs += [(qg, h, kp) for h in range(NH)
                             for kp in range(NKV // 2)]
                ei = 0
                pv_tiles = {}
                yn_tiles = {}
                todo = []   # (trigger_idx, fn)

                def emit_q(ng):
                    qs = slice(ng * 512, (ng + 1) * 512)
                    psq = bpsum.tile([128, 512], F32, tag="sc", bufs=3,
                                     name=f"psqq{ng}")
                    for k in range(KE):
                        nc.tensor.matmul(psq[:],
                                         wqkT_sb[:, k, 0:128],
                                         xts[ng][:, k, :], start=(k == 0),
                                         stop=(k == KE - 1))
                    nc.scalar.activation(qk_sb[:, 0, 0, qs], psq[:],
                                         AF.Identity, bias=bq_sb[:, 0:1])

                for ng in range(NQG):
                    load_x(ng)
                chain_q = []
                for ng in range(NQG):
                    chain_q += [(ng, c) for c in kv_chains(ng)]
                kv_done = [0]

                def drain_chain():
                    if chain_q:
                        ng, c = chain_q.pop(0)
                        c()
                        if not chain_q or chain_q[0][0] != ng:
                            kv_done[0] = ng + 1

                def need_kv(ng_k):
                    while kv_done[0] <= ng_k:
                        drain_chain()

                def emit_scores(qg, h, kp):
                    need_kv((2 * kp + 1) // 4)
                    sc = bpsum.tile([128, 2, 512], F32, tag="sc", bufs=3,
                                    name=f"sc{qg}_{h}_{kp}")
                    for jj in range(2):
                        lhsT, rhs = qk_aps(h, 2 * kp + jj, qg)
                        nc.tensor.matmul(sc[:, jj, :], lhsT, rhs,
                                         start=True, stop=True, perf_mode=DR)
                    return sc

                def emit_exp(sc, qg, h, kp):
                    nonlocal ei
                    pt = spool.tile([128, 2, 512], F8, tag="p", bufs=6,
                                    name=f"p{qg}_{h}_{kp}")
                    e = EXP_PATTERN[ei % len(EXP_PATTERN)]
                    ei += 1
                    if e == "A":
                        nc.scalar.activation(pt[:], sc[:], AF.Exp, scale=0.125)
                    else:
                        nc.vector.tensor_scalar(pt.bitcast(I8), sc[:],
                                                EXPA, EXPB, ALU.mult, ALU.add)
                    return pt

                def emit_recip_bcast(qg, h):
                    pv = pv_tiles[(qg, h)]
                    rcp = spool.tile([1, 512], F32, tag="rcp", bufs=2,
                                     name=f"rcp{qg}_{h}")
                    nc.vector.reciprocal(rcp[:], pv[64:65, :])
                    rb = spool.tile([64, 512], F32, tag="rb", bufs=2,
                                    name=f"rb{qg}_{h}")
                    nc.gpsimd.partition_broadcast(rb[:], rcp[:])
                    return rb

                def emit_mul(qg, h, rb):
                    pv = pv_tiles[(qg, h)]
                    yn = spool.tile([64, 512], BF, tag="yn", bufs=6,
                                    name=f"yn{qg}_{h}")
                    nc.vector.tensor_tensor(yn[:], pv[0:64, :], rb[:],
                                            ALU.mult)
                    yn_tiles[(qg, h)] = yn

                def emit_proj(qg):
                    for f in range(2):
                        fw = 512 if f == 0 else E - 512
                        fsl = slice(f * 512, f * 512 + fw)
                        for qb in range(4):
                            pp = bpsum.tile([128, fw], F32, tag="acc",
                                            bufs=2, name=f"pp{qg}_{f}_{qb}")
                            for h in range(NH):
                                nc.tensor.matmul(
                                    pp[:],
                                    yn_tiles[(qg, h)][:, qb * 128:(qb + 1) * 128],
                                    wpT_sb[:, h, fsl],
                                    start=(h == 0), stop=(h == NH - 1))
                            ost = spool.tile([128, fw], BF, tag="ost", bufs=4,
                                             name=f"ost{qg}_{f}_{qb}")
                            nc.scalar.copy(ost[:], pp[:])
                            nc.sync.dma_start(
                                out[qg * 512 + qb * 128:
                                    qg * 512 + (qb + 1) * 128, fsl], ost[:])
                    if qg + 2 < NQG:
                        emit_q(qg + 2)

                def norm_closure(qg, h, idx):
                    d1, d2 = (1, 3) if h < 2 else (2, 4)
                    def stage1():
                        rb = emit_recip_bcast(qg, h)
                        todo.append((idx + d2, lambda: emit_mul(qg, h, rb)))
                    todo.append((idx + d1, stage1))
                    if h == NH - 1:
                        todo.append((idx + 6, lambda: emit_proj(qg)))

                need_kv(0)
                emit_q(0)
                todo.append((32, lambda: emit_q(1)))
                pending = [emit_scores(*jobs[i]) for i in range(DEPTH)]
                for idx, (qg, h, kp) in enumerate(jobs):
                    sc = pending.pop(0)
                    pt = emit_exp(sc, qg, h, kp)
                    if idx + DEPTH < len(jobs):
                        pending.append(emit_scores(*jobs[idx + DEPTH]))
                    if kp == 0:
                        pv = bpsum.tile([65, 512], F32, tag="acc", bufs=2,
                                        name=f"pv{qg}_{h}")
                        pv_tiles[(qg, h)] = pv[:]
                    nc.tensor.matmul(pv_tiles[(qg, h)],
                                     v_sb[:, 2 * kp:2 * kp + 2, h, 0:HD + 1],
                                     pt[:], start=(kp == 0),
                                     stop=(kp == NKV // 2 - 1), perf_mode=DR)
                    drain_chain()
                    todo.sort(key=lambda t: t[0])
                    while todo and idx >= todo[0][0]:
                        todo.pop(0)[1]()
                    if kp == NKV // 2 - 1:
                        norm_closure(qg, h, idx)
                while todo:
                    todo.sort(key=lambda t: t[0])
                    todo.pop(0)[1]()

    nc.finalize()
    return nc


def host_prep(x, w_qkv, b_qkv, w_proj, b_proj, n_tokens=N):
    """Build per-core input maps + the host-side combine closure."""
    x = np.asarray(x, np.float32)
    w_qkv = np.asarray(w_qkv, np.float32)
    b_qkv = np.asarray(b_qkv, np.float32)
    w_proj = np.asarray(w_proj, np.float32)
    b_proj = np.asarray(b_proj, np.float32)

    xT = [np.ascontiguousarray(x[b].T).astype(ml_dtypes.bfloat16)
          for b in range(B)]

    in_maps = []
    for c in range(8):
        b, g = divmod(c, M_GROUPS)
        base = g * NH * 3 * HD  # row offset of this group in w_qkv
        wq = [w_qkv[base + i * 3 * HD: base + i * 3 * HD + HD]
              for i in range(NH)]
        wk = [w_qkv[base + i * 3 * HD + HD: base + i * 3 * HD + 2 * HD]
              for i in range(NH)]
        wv = [w_qkv[base + i * 3 * HD + 2 * HD: base + i * 3 * HD + 3 * HD]
              for i in range(NH)]
        bqv = [b_qkv[base + i * 3 * HD: base + i * 3 * HD + HD]
               for i in range(NH)]
        # m-tiles: m0=[Q0;Q1], m1=[K0;K1], m2=[Q2;K2]
        wqkT = np.concatenate(
            [wq[0], wq[1], wk[0], wk[1], wq[2], wk[2]], axis=0).T  # [E, 384]
        wvT = np.concatenate(wv, axis=0).T  # [E, 192]
        bqa = np.zeros((2, 128), np.float32)
        bqa[0, 0:HD] = bqv[0]
        bqa[0, HD:2 * HD] = bqv[1]
        bqa[1, 0:HD] = bqv[2]
        wp = w_proj[:, g * GD:(g + 1) * GD]  # [768, 192]
        wpT = np.ascontiguousarray(
            wp.T.reshape(NH, HD, E).transpose(1, 0, 2))  # [64, 3, 768]
        in_maps.append({
            "xT": np.ascontiguousarray(xT[b]),
            "wqkT": np.ascontiguousarray(wqkT).astype(ml_dtypes.bfloat16),
            "wvT": np.ascontiguousarray(wvT).astype(ml_dtypes.bfloat16),
            "bq": bqa,
            "wpT": wpT.astype(ml_dtypes.bfloat16),
        })

    bv_all = np.concatenate(
        [b_qkv[h * 3 * HD + 2 * HD: (h + 1) * 3 * HD] for h in range(H)])
    b_eff = b_proj + w_proj @ bv_all

    def combine(results):
        o = np.empty((B, n_tokens, E), np.float32)
        for b in range(B):
            acc = results[b * M_GROUPS]["out"].astype(np.float32)
            for g in range(1, M_GROUPS):
                acc = acc + results[b * M_GROUPS + g]["out"].astype(np.float32)
            o[b] = acc + b_eff
        return o

    return in_maps, combine


_NC_CACHE = {}


def kernel(x, w_qkv, b_qkv, w_proj, b_proj):
    if "nc" not in _NC_CACHE:
        _NC_CACHE["nc"] = build_nc()
    nc = _NC_CACHE["nc"]
    in_maps, combine = host_prep(x, w_qkv, b_qkv, w_proj, b_proj)
    res = run_bass_kernel_spmd(nc, in_maps, core_ids=list(range(8)))
    return combine(res.results)


if __name__ == "__main__":
    rng = np.random.default_rng(0)
    inputs = {
        "x": rng.normal(size=(B, N, E)).astype(np.float32),
        "w_qkv": (rng.normal(size=(3 * E, E)) * 0.02).astype(np.float32),
        "b_qkv": (rng.normal(size=(3 * E,)) * 0.02).astype(np.float32),
        "w_proj": (rng.normal(size=(E, E)) * 0.02).astype(np.float32),
        "b_proj": (rng.normal(size=(E,)) * 0.02).astype(np.float32),
    }
    o = kernel(**inputs)
    print("out", o.shape, o.dtype, float(np.abs(o).mean()))
